# revision 12
# baseline (speedup 1.0000x reference)
"""MoE routing kernel for Trainium2 (8 NeuronCores, data-parallel over batch).

Reference computation (B=1024, PHASE=64, GATE=128, K=8, D=512):
    coeff = softmax(gateMLP(phase))                       # [B, K]
    per layer l in 0..2:
        y = sum_k coeff[:,k] * (y @ W_l[k]) + coeff @ b_l # [B, D]
        y = elu(y)  (layers 0,1 only)

Device mapping (per core, B_local = 128 rows):
  - Gate runs in transposed-activation layout (h.T = [g, b]) so no input
    transposes are needed; phase.T is prepared on the host.
  - Expert mixing: z_k.T = y.T * coeff[:,k] is computed on the TensorE as
    y_chunk.T @ diag(coeff[:,k]) with 4 experts' diagonals concatenated in
    one N=512 matmul (full fp32r rate), which fuses the transpose AND the
    per-sample scaling.
  - Main matmuls accumulate sum_k z_k @ W_l[k] over 32 matmuls in one PSUM
    bank; the mixed bias coeff @ b_l is one extra matmul (contraction K=8).
  - ELU is exact: elu(x) = relu(x) + (exp(min(x, 0)) - 1).
"""

import numpy as np

import concourse.bass as bass
import concourse.mybir as mybir
import concourse.tile as tile
from concourse import bacc

AFT = mybir.ActivationFunctionType
ALU = mybir.AluOpType
F32 = mybir.dt.float32
F32R = mybir.dt.float32r
F16 = mybir.dt.float16
AX = mybir.AxisListType

B, PHASE, GATE, K, D = 1024, 64, 128, 8, 512
NCORES = 8
BL = B // NCORES          # 128 rows per core
IC = D // 128             # 4 contraction chunks of 128

# Precision of the expert-path matmuls (weights, mixed activations z, x).
#   "f16": fp16 operands — full TensorE rate (1 cyc/row) and half the weight
#          DMA traffic. PSUM accumulation stays fp32.
#   "f32": exact fp32 operands — quarter TensorE rate, 2x weight DMA.
# (fp32r exists but walrus requires operands pre-rounded to fp32r and it is
# reduced-precision anyway, so fp16 dominates it here.)
W_MODE = "f16"


def _w_dt():
    return F16 if W_MODE == "f16" else F32


def emit_moe(tc, out_ap, ins):
    """Emit the per-core MoE program. ins is a dict of DRAM APs."""
    nc = tc.nc
    w_dt = _w_dt()

    with (
        tc.tile_pool(name="consts", bufs=1) as cpool,
        tc.tile_pool(name="wpool", bufs=2) as wpool,
        tc.tile_pool(name="ypool", bufs=2) as ypool,
        tc.tile_pool(name="zpool", bufs=2) as zpool,
        tc.tile_pool(name="tmp", bufs=3) as tpool,
        tc.tile_pool(name="ps_out", bufs=2, space="PSUM") as ps_out,
        tc.tile_pool(name="ps_z", bufs=6, space="PSUM") as ps_z,
    ):
        # ---- constant / input loads -------------------------------------
        t_ident = cpool.tile([128, 128], F32)
        nc.sync.dma_start(out=t_ident, in_=ins["ident"])
        t_x = cpool.tile([BL, D], w_dt)
        nc.sync.dma_start(out=t_x, in_=ins["x"])
        t_phT = cpool.tile([PHASE, BL], F32)
        nc.sync.dma_start(out=t_phT, in_=ins["phT"])
        t_gw0 = cpool.tile([PHASE, GATE], F32)
        nc.sync.dma_start(out=t_gw0, in_=ins["gw0"])
        t_gw1 = cpool.tile([GATE, GATE], F32)
        nc.sync.dma_start(out=t_gw1, in_=ins["gw1"])
        t_gw2 = cpool.tile([GATE, K], F32)
        nc.sync.dma_start(out=t_gw2, in_=ins["gw2"])
        t_gb0 = cpool.tile([GATE, 1], F32)
        nc.sync.dma_start(out=t_gb0, in_=ins["gb0"])
        t_gb1 = cpool.tile([GATE, 1], F32)
        nc.sync.dma_start(out=t_gb1, in_=ins["gb1"])
        t_gb2 = cpool.tile([1, K], F32)
        nc.sync.dma_start(out=t_gb2, in_=ins["gb2"])
        t_bias = cpool.tile([K, 3 * D], w_dt)  # the 3 layers' expert biases
        nc.sync.dma_start(out=t_bias, in_=ins["eb"])

        t_ones = cpool.tile([1, GATE], F32)
        nc.vector.memset(t_ones, 1.0)

        # ---- gate MLP (transposed-activation layout) --------------------
        # h1.T[g, b] = elu(gw0.T @ phase.T + gb0)
        p_g = ps_z.tile([128, 512], F32, tag="zps")
        nc.tensor.matmul(p_g[:GATE, :BL], lhsT=t_gw0, rhs=t_phT, start=True, stop=True)
        h1 = tpool.tile([GATE, BL], F32, tag="h")
        _elu(nc, tpool, h1, p_g[:GATE, :BL], bias=t_gb0)

        p_g2 = ps_z.tile([128, 512], F32, tag="zps")
        nc.tensor.matmul(p_g2[:GATE, :BL], lhsT=t_gw1, rhs=h1, start=True, stop=True)
        h2 = tpool.tile([GATE, BL], F32, tag="h")
        _elu(nc, tpool, h2, p_g2[:GATE, :BL], bias=t_gb1)

        # logits[b, k] = h2.T.T @ gw2 + 1.T @ gb2  (normal layout)
        p_lg = ps_z.tile([128, 512], F32, tag="zps")
        nc.tensor.matmul(p_lg[:BL, :K], lhsT=h2, rhs=t_gw2, start=True, stop=False)
        nc.tensor.matmul(p_lg[:BL, :K], lhsT=t_ones, rhs=t_gb2, start=False, stop=True)

        # softmax over K (free dim)
        t_mx = tpool.tile([BL, 1], F32)
        nc.vector.reduce_max(t_mx, p_lg[:BL, :K], axis=AX.X)
        t_nmx = tpool.tile([BL, 1], F32)
        nc.vector.tensor_scalar_mul(t_nmx, t_mx, -1.0)
        t_exp = tpool.tile([BL, K], F32)
        nc.scalar.activation(t_exp, p_lg[:BL, :K], AFT.Exp, bias=t_nmx, scale=1.0)
        t_sum = tpool.tile([BL, 1], F32)
        nc.vector.reduce_sum(t_sum, t_exp, axis=AX.X)
        t_rcp = tpool.tile([BL, 1], F32)
        nc.vector.reciprocal(t_rcp, t_sum)
        t_coeff = cpool.tile([BL, K], F32)
        nc.vector.tensor_scalar_mul(t_coeff, t_exp, t_rcp)

        # coeff.T via PE transpose (for the mixed-bias matmul)
        p_ct = ps_z.tile([128, 512], F32, tag="zps")
        nc.tensor.transpose(p_ct[:K, :BL], t_coeff, t_ident)
        t_coeffT = cpool.tile([K, BL], w_dt)
        nc.scalar.copy(t_coeffT, p_ct[:K, :BL])

        # diag quads: [diag(c_{4q}) | diag(c_{4q+1}) | diag(c_{4q+2}) | diag(c_{4q+3})]
        t_diag = cpool.tile([128, 2 * 512], w_dt)
        for k in range(K):
            nc.vector.tensor_scalar_mul(
                t_diag[:, k * 128:(k + 1) * 128], t_ident, t_coeff[:, k:k + 1]
            )

        # ---- main expert layers -----------------------------------------
        y = t_x
        for l in range(3):
            # weights for this layer: [128, k*2048 + ic*512 + o]
            t_w = wpool.tile([128, K * IC * D], w_dt, tag="w")
            for k in range(K):
                nc.sync.dma_start(
                    out=t_w[:, k * 2048:(k + 1) * 2048], in_=ins["W"][l, k]
                )

            # step A: z_k.T = y.T * coeff[:,k], 4 experts per matmul
            t_z = zpool.tile([128, K * D], w_dt, tag="z")
            for q in range(2):
                for ic in range(IC):
                    p_z = ps_z.tile([128, 512], F32, tag="zps")
                    nc.tensor.matmul(
                        p_z,
                        lhsT=y[:, ic * 128:(ic + 1) * 128],
                        rhs=t_diag[:, q * 512:(q + 1) * 512],
                        start=True,
                        stop=True,
                    )
                    # scatter into z layout [p, k*512 + ic*128 + c]:
                    # psum column kq*128+c holds z_{4q+kq}.T[ic*128+p, c]
                    dst = t_z.rearrange("p (k i c) -> p k i c", k=K, i=IC)[
                        :, 4 * q:4 * q + 4, ic:ic + 1, :
                    ]
                    src = p_z.rearrange("p (k i c) -> p k i c", k=4, i=1)
                    nc.vector.tensor_copy(out=dst, in_=src)

            # step B: out = sum_k z_k @ W_l[k] + coeff @ b_l
            p_o = ps_out.tile([BL, D], F32, tag="out")
            for k in range(K):
                for ic in range(IC):
                    nc.tensor.matmul(
                        p_o,
                        lhsT=t_z[:, k * 512 + ic * 128:k * 512 + (ic + 1) * 128],
                        rhs=t_w[:, k * 2048 + ic * 512:k * 2048 + (ic + 1) * 512],
                        start=(k == 0 and ic == 0),
                        stop=False,
                    )
            nc.tensor.matmul(
                p_o,
                lhsT=t_coeffT,
                rhs=t_bias[:, l * D:(l + 1) * D],
                start=False,
                stop=True,
            )

            if l < 2:
                y_next = ypool.tile([BL, D], w_dt, tag="y")
                _elu(nc, tpool, y_next, p_o, bias=None)
                y = y_next
            else:
                t_out = ypool.tile([BL, D], F32, tag="y")
                nc.scalar.copy(t_out, p_o)
                nc.sync.dma_start(out=out_ap, in_=t_out)


def _elu(nc, tpool, out, pre, bias):
    """out = elu(pre + bias); pre may live in PSUM. Exact:
    elu(x) = relu(x) + exp(min(x, 0)) - 1."""
    shape = [pre.partition_size(), pre.free_size()]
    t_m = tpool.tile(shape, F32, tag="elu_m")
    if bias is None:
        nc.vector.tensor_scalar_min(t_m, pre, 0.0)
    else:
        nc.vector.tensor_scalar(t_m, pre, bias, 0.0, op0=ALU.add, op1=ALU.min)
    t_e = tpool.tile(shape, F32, tag="elu_e")
    nc.scalar.activation(t_e, t_m, AFT.Exp)
    t_r = tpool.tile(shape, F32, tag="elu_r")
    nc.scalar.activation(t_r, pre, AFT.Relu, bias=(0.0 if bias is None else bias), scale=1.0)
    # out = (e - 1) + r
    nc.vector.scalar_tensor_tensor(
        out, in0=t_e, scalar=1.0, in1=t_r, op0=ALU.subtract, op1=ALU.add
    )


def _prep_host(x, phase, gw0, gb0, gw1, gb1, gw2, gb2, W0, b0, W1, b1, W2, b2):
    """Host-side packing. Returns (shared_map, per_core_maps)."""
    f32 = np.float32
    w_np = np.float16 if W_MODE == "f16" else f32

    # weights blob: [3, 8, 128, 2048]; [l, k, p, ic*512 + o] = W_l[k, ic*128+p, o]
    W = np.stack([W0, W1, W2]).astype(f32)  # [3, 8, 512, 512]
    Wb = (
        W.reshape(3, K, IC, 128, D)
        .transpose(0, 1, 3, 2, 4)
        .reshape(3, K, 128, IC * D)
        .astype(w_np)
    )
    eb = np.concatenate([b0, b1, b2], axis=1).astype(w_np)  # [8, 1536]

    shared = {
        "ident": np.eye(128, dtype=f32),
        "gw0": np.ascontiguousarray(gw0.astype(f32)),
        "gw1": np.ascontiguousarray(gw1.astype(f32)),
        "gw2": np.ascontiguousarray(gw2.astype(f32)),
        "gb0": np.ascontiguousarray(gb0.astype(f32).reshape(GATE, 1)),
        "gb1": np.ascontiguousarray(gb1.astype(f32).reshape(GATE, 1)),
        "gb2": np.ascontiguousarray(gb2.astype(f32).reshape(1, K)),
        "W": np.ascontiguousarray(Wb),
        "eb": np.ascontiguousarray(eb),
    }
    per_core = []
    for c in range(NCORES):
        sl = slice(c * BL, (c + 1) * BL)
        m = dict(shared)
        m["x"] = np.ascontiguousarray(x[sl].astype(w_np))
        m["phT"] = np.ascontiguousarray(phase[sl].astype(f32).T)
        per_core.append(m)
    return per_core


def _declare_dram(nc):
    f32 = mybir.dt.float32
    w_dt = _w_dt()
    ins = {
        "ident": nc.dram_tensor("ident", [128, 128], f32, kind="ExternalInput").ap(),
        "x": nc.dram_tensor("x", [BL, D], w_dt, kind="ExternalInput").ap(),
        "phT": nc.dram_tensor("phT", [PHASE, BL], f32, kind="ExternalInput").ap(),
        "gw0": nc.dram_tensor("gw0", [PHASE, GATE], f32, kind="ExternalInput").ap(),
        "gw1": nc.dram_tensor("gw1", [GATE, GATE], f32, kind="ExternalInput").ap(),
        "gw2": nc.dram_tensor("gw2", [GATE, K], f32, kind="ExternalInput").ap(),
        "gb0": nc.dram_tensor("gb0", [GATE, 1], f32, kind="ExternalInput").ap(),
        "gb1": nc.dram_tensor("gb1", [GATE, 1], f32, kind="ExternalInput").ap(),
        "gb2": nc.dram_tensor("gb2", [1, K], f32, kind="ExternalInput").ap(),
        "W": nc.dram_tensor("W", [3, K, 128, IC * D], w_dt, kind="ExternalInput").ap(),
        "eb": nc.dram_tensor("eb", [K, 3 * D], w_dt, kind="ExternalInput").ap(),
    }
    out = nc.dram_tensor("out", [BL, D], f32, kind="ExternalOutput").ap()
    return ins, out


_CACHED = None


def _build():
    global _CACHED
    if _CACHED is None:
        nc = bacc.Bacc(
            "TRN2", target_bir_lowering=False, debug=False, num_devices=NCORES
        )
        ins, out = _declare_dram(nc)
        with tile.TileContext(nc) as tc:
            emit_moe(tc, out, ins)
        nc.compile()
        _CACHED = nc
    return _CACHED


def kernel(**inputs) -> np.ndarray:
    from concourse.bass_utils import run_bass_kernel_spmd

    per_core = _prep_host(**inputs)
    nc = _build()
    res = run_bass_kernel_spmd(nc, per_core, core_ids=list(range(NCORES)))
    return np.concatenate([r["out"] for r in res.results], axis=0)


if __name__ == "__main__":
    import reference

    inp = {k: np.asarray(v) for k, v in reference.setup_inputs().items()}
    got = kernel(**inp)
    exp = np.asarray(reference.reference(**inp))
    err = np.abs(got - exp).max() / np.abs(exp).max()
    print("Relative error:", err)
